# revision 29
# baseline (speedup 1.0000x reference)
"""GQA attention kernel for 8 Trainium2 NeuronCores.

Sharding: core c -> (b = c // 4, kv-group gk = c % 4).
Each core computes, for its batch b and its kv head gk (which owns the 4
contiguous q-heads gk*4..gk*4+3):
    q/k/v projections, attention, and a partial out-projection
    out_partial[b] = o_heads @ Wo[:, gk*512:(gk+1)*512].T
Host sums the 4 partials per batch.

All matmuls in bf16 (fp32 PSUM accumulation). Softmax without max
subtraction (scores are bounded ~|4.5| at this problem's weight scale);
row sums come free from a ones-column appended to V; normalization is
applied to the 128-wide per-head output ahead of the out projection.

Schedule: the scalar engine's exp stream (~133us) is the scarce
co-resource, so the score chunks start as early as possible and the
q-projection chains, v-projection chains and out-projection tiles are
used as PE filler between score matmuls throughout the 16-chunk loop.
Input DMAs issue from the gpsimd queue (idle otherwise) so the next
iteration's loads prefetch as soon as buffers free; output stores issue
from the sync queue.

Layout (per core), everything E/K-major for the PE:
  xT  [E, N]   = x[b].T          kT [128d, N]    scoresT [s, n] chunks
  wq  [E, 512] = Wq rows.T       qT [128, 4g, N]
  wk  [E, 128] = Wk rows.T       v  [128, 16st, 130] (col 128 = ones)
  wv  [E, 128]                   oT [128, 4g, N]
  wo  [512, E] = Wo cols.T       out [N, E] bf16 partial (host sums in f32)
"""

import sys

sys.path.insert(0, "/opt/trn_rl_repo")

import numpy as np
import ml_dtypes

import concourse.bass as bass
import concourse.mybir as mybir
import concourse.tile as tile
from concourse import bacc
from concourse.bass_utils import run_bass_kernel_spmd
from concourse.masks import make_identity

BF16 = mybir.dt.bfloat16
F32 = mybir.dt.float32
bf16 = ml_dtypes.bfloat16

B, N, E = 2, 2048, 2048
H, D, G = 16, 128, 4
HKV = H // G
JL = G * D                     # 512 local q-head dims per core
ET = E // 128                  # 16
NT = N // 128                  # 16
CH = N // 512                  # 4
SCALE = 1.0 / float(np.sqrt(D))

_cached = {}


def _build(iters=1):
    nc = bacc.Bacc("TRN2", target_bir_lowering=False, debug=False, num_devices=8)

    xT = nc.dram_tensor("xT", [E, N], BF16, kind="ExternalInput")
    wq = nc.dram_tensor("wq", [E, JL], BF16, kind="ExternalInput")
    wk = nc.dram_tensor("wk", [E, D], BF16, kind="ExternalInput")
    wv = nc.dram_tensor("wv", [E, D], BF16, kind="ExternalInput")
    wo = nc.dram_tensor("wo", [JL, E], BF16, kind="ExternalInput")
    out = nc.dram_tensor("out", [N, E], BF16, kind="ExternalOutput")

    with tile.TileContext(nc) as tc:
        with (
            tc.tile_pool(name="const", bufs=1) as cpool,
            tc.tile_pool(name="xp", bufs=1) as xpool,
            tc.tile_pool(name="wp", bufs=1) as wpool,
            tc.tile_pool(name="kvp", bufs=1) as kvpool,
            tc.tile_pool(name="qp", bufs=1) as qpool,
            tc.tile_pool(name="pp", bufs=2) as ppool,
            tc.tile_pool(name="op", bufs=4) as opool,
            tc.tile_pool(name="otp", bufs=1) as otpool,
            tc.tile_pool(name="outp", bufs=3) as outpool,
            tc.tile_pool(name="psS", bufs=2, space="PSUM") as PS,   # scores (2x2 banks)
            tc.tile_pool(name="psB", bufs=2, space="PSUM") as PB,   # k/q/out chains + transpose
            tc.tile_pool(name="psO", bufs=2, space="PSUM") as PO,   # v/av
        ):
            ident = cpool.tile([128, 128], BF16, tag="ident")
            make_identity(nc, ident[:])

            for _ in range(iters):
                _emit_iter(nc, tc, ident, xpool, wpool, kvpool, qpool, ppool,
                           opool, otpool, outpool, PS, PB, PO,
                           xT, wq, wk, wv, wo, out)

    nc.compile()
    return nc


def _emit_iter(nc, tc, ident, xpool, wpool, kvpool, qpool, ppool, opool,
               otpool, outpool, PS, PB, PO, xT, wq, wk, wv, wo, out):
    x_sb = xpool.tile([128, ET, N], BF16, tag="x")
    wq_sb = wpool.tile([128, ET, JL], BF16, tag="wq")
    wk_sb = wpool.tile([128, ET, D], BF16, tag="wk")
    wv_sb = wpool.tile([128, ET, D], BF16, tag="wv")
    wo_sb = wpool.tile([128, G, E], BF16, tag="wo")
    kT_sb = kvpool.tile([128, N], BF16, tag="kT")
    v_sb = kvpool.tile([128, NT, 130], BF16, tag="v")
    qT_sb = qpool.tile([128, G, N], BF16, tag="qT")
    oT_sb = otpool.tile([128, G, N], BF16, tag="oT")

    # --- input DMAs, all on the gpsimd queue (otherwise idle) so the next
    # iteration's loads start as soon as each buffer frees.
    nc.gpsimd.dma_start(wk_sb[:], wk.rearrange("(a p) d -> p a d", p=128))
    nc.gpsimd.dma_start(wv_sb[:], wv.rearrange("(a p) d -> p a d", p=128))
    xr = xT.rearrange("(a p) n -> p a n", p=128)
    for i in range(8):
        nc.gpsimd.dma_start(x_sb[:, 2 * i:2 * i + 2, :], xr[:, 2 * i:2 * i + 2, :])
    wqr = wq.rearrange("(a p) j -> p a j", p=128)
    for i in range(2):
        nc.gpsimd.dma_start(wq_sb[:, 8 * i:8 * i + 8, :], wqr[:, 8 * i:8 * i + 8, :])
    for jt in range(G):
        nc.gpsimd.dma_start(wo_sb[:, jt, :], wo[jt * 128:(jt + 1) * 128, :])

    nc.vector.memset(v_sb[:, :, 128:129], 1.0)

    # ---------- emit helpers ----------
    def emit_k_chain(sc):
        ps = PB.tile([128, 512], F32, tag="b")
        for et in range(ET):
            nc.tensor.matmul(ps, wk_sb[:, et, :],
                             x_sb[:, et, sc * 512:(sc + 1) * 512],
                             start=(et == 0), stop=(et == ET - 1))
        nc.vector.tensor_copy(kT_sb[:, sc * 512:(sc + 1) * 512], ps)

    def emit_q_half(g, c, cell, half):
        """Half of a q-projection chain (8 of 16 accumulation steps) so q
        chains can interleave as ~1.7us PE filler quanta. The psum tile is
        allocated at first emission so pool rotation follows PE order."""
        if half == 0:
            cell.append(PB.tile([128, 512], F32, tag="b", name=f"q{g}_{c}"))
        ps = cell[0]
        for et in range(8 * half, 8 * half + 8):
            nc.tensor.matmul(ps, wq_sb[:, et, g * 128:(g + 1) * 128],
                             x_sb[:, et, c * 512:(c + 1) * 512],
                             start=(et == 0), stop=(et == ET - 1))
        if half == 1:
            nc.vector.tensor_copy(qT_sb[:, g, c * 512:(c + 1) * 512], ps)

    def emit_q_chain(g, c):
        cell = []
        emit_q_half(g, c, cell, 0)
        emit_q_half(g, c, cell, 1)

    def emit_v_chain(st):
        ps = PO.tile([128, 130], F32, tag="oc")
        for et in range(ET):
            nc.tensor.matmul(ps[:, 0:128], x_sb[:, et, st * 128:(st + 1) * 128],
                             wv_sb[:, et, :],
                             start=(et == 0), stop=(et == ET - 1))
        nc.vector.tensor_copy(v_sb[:, st, 0:128], ps[:, 0:128])

    def emit_score_pair(g, c, p_t, sp):
        """Scores for s-tiles 2sp,2sp+1 into one 2-bank psum tile; a single
        1024-wide exp (the scalar engine's per-instruction overhead is
        ~100ns, so wider activations are cheaper per element)."""
        ps = PS.tile([128, 1024], F32, tag="s")
        for half in range(2):
            st = 2 * sp + half
            nc.tensor.matmul(ps[:, half * 512:(half + 1) * 512],
                             kT_sb[:, st * 128:(st + 1) * 128],
                             qT_sb[:, g, c * 512:(c + 1) * 512],
                             start=True, stop=True)
        nc.scalar.activation(p_t[:, 2 * sp * 512:(2 * sp + 2) * 512], ps,
                             mybir.ActivationFunctionType.Exp, scale=SCALE)

    # The transpose of each o-group is deferred into the next o-quantum so
    # the PE doesn't wait on the DVE normalize-multiply that feeds it.
    # (XBAR DMA transposes were tried instead and regressed: the sync DMA
    # path is bandwidth-critical for stores and the scalar sequencer is
    # throughput-critical for exps.)
    tp_defer = []

    def flush_tp():
        while tp_defer:
            tp_defer.pop(0)()

    def emit_o_group(g, c, p_t, t):
        flush_tp()
        pso = PO.tile([128, 130], F32, tag="oc")
        for st in range(NT):
            nc.tensor.matmul(
                pso[:, 0:129], p_t[:, st * 512 + t * 128: st * 512 + (t + 1) * 128],
                v_sb[:, st, 0:129],
                start=(st == 0), stop=(st == NT - 1),
            )
        rc = opool.tile([128, 1], F32, tag="recip")
        nc.vector.reciprocal(rc[:], pso[:, 128:129])
        o_n = opool.tile([128, 128], BF16, tag="o_n")
        nc.vector.tensor_scalar_mul(o_n[:], pso[:, 0:128], rc[:])

        def tp():
            pst = PB.tile([128, 128], BF16, tag="b")
            nc.tensor.transpose(pst[:], o_n[:], ident[:])
            nc.vector.tensor_copy(
                oT_sb[:, g, c * 512 + t * 128: c * 512 + (t + 1) * 128], pst[:],
            )
        tp_defer.append(tp)

    def make_out_nt(nt):
        """Return the 4 per-512-col out-projection chain emitters for row
        tile nt, plus the closing DMA. Chains can be interleaved as PE
        filler; call close() after the last chain."""
        stage = outpool.tile([128, 2048], BF16, tag="out")

        def chain(ec):
            ps = PB.tile([128, 512], F32, tag="b")
            for g in range(G):
                nc.tensor.matmul(ps, oT_sb[:, g, nt * 128:(nt + 1) * 128],
                                 wo_sb[:, g, ec * 512:(ec + 1) * 512],
                                 start=(g == 0), stop=(g == G - 1))
            nc.vector.tensor_copy(stage[:, ec * 512:(ec + 1) * 512], ps)

        def close():
            nc.sync.dma_start(out[nt * 128:(nt + 1) * 128, :], stage[:])

        return chain, close

    # ---------- schedule ----------
    # Pre-loop: kT (4 chains) + q for chunk 0.
    for sc in range(CH):
        emit_k_chain(sc)
    emit_q_chain(0, 0)

    chunks = [(c, g) for c in range(CH) for g in range(G)]
    # q lookahead: chunk0 computes q(chunk1); chunks 1..7 compute two each.
    qsched = {0: [chunks[1]]}
    nxt = 2
    for i in range(1, 8):
        qsched[i] = chunks[nxt:nxt + 2]
        nxt += 2
    # out rows: region r (rows 4r..4r+3) ready after AV of chunk 4r+3,
    # which runs during chunk 4r+4 -> nt 4r at end of chunk 4r+4, then one
    # per chunk; nts 12..15 go in the tail.
    outsched = {}
    for r in range(3):
        for j in range(4):
            outsched[4 * r + 4 + j] = 4 * r + j

    prev = None
    for i, (c, g) in enumerate(chunks):
        p_t = ppool.tile([128, NT * 512], BF16, tag="p", name=f"p{i}")

        # Filler quanta (each ~0.7-1.7us of PE work) emitted one per score
        # pair so the PE never stalls on the double-buffered score psum
        # rotation while the scalar engine drains exps. o-groups first (out
        # rows of this chunk's region may depend on them), then out chains,
        # then q halves.
        fill = []
        if prev is not None:
            pg, pc, pp = prev
            for sub in range(4):
                fill.append(lambda sub=sub, pg=pg, pc=pc, pp=pp:
                            emit_o_group(pg, pc, pp, sub))
        if i == 0:
            for st2 in range(0, NT, 2):
                fill.append(lambda st2=st2: (emit_v_chain(st2),
                                             emit_v_chain(st2 + 1)))
        out_nt = outsched.get(i)
        if out_nt is not None:
            ochain, oclose = make_out_nt(out_nt)
            for ec in range(4):
                fill.append(lambda ec=ec, ochain=ochain: ochain(ec))
            fill.append(oclose)
        for (qc, qg) in qsched.get(i, []):
            cell = []
            for half in range(2):
                fill.append(lambda qg=qg, qc=qc, cell=cell, half=half:
                            emit_q_half(qg, qc, cell, half))

        for sp in range(NT // 2):
            emit_score_pair(g, c, p_t, sp)
            if fill:
                fill.pop(0)()
        while fill:
            fill.pop(0)()
        prev = (g, c, p_t)

    # Tail: last chunk's AV interleaved with out rows 12..15.
    for sub in range(4):
        emit_o_group(prev[0], prev[1], prev[2], sub)
        flush_tp()
        chain, close = make_out_nt(12 + sub)
        for ec in range(4):
            chain(ec)
        close()


def get_nc(iters=1):
    key = ("nc", iters)
    if key not in _cached:
        _cached[key] = _build(iters)
    return _cached[key]


def make_in_maps(x, Wq, Wk, Wv, Wo):
    """Per-core host-side sharding. Core c -> (b=c//4, gk=c%4)."""
    in_maps = []
    xT = [np.ascontiguousarray(x[b].T).astype(bf16) for b in range(B)]
    wq_s = [np.ascontiguousarray(Wq[gk * JL:(gk + 1) * JL, :].T).astype(bf16)
            for gk in range(HKV)]
    wk_s = [np.ascontiguousarray(Wk[gk * D:(gk + 1) * D, :].T).astype(bf16)
            for gk in range(HKV)]
    wv_s = [np.ascontiguousarray(Wv[gk * D:(gk + 1) * D, :].T).astype(bf16)
            for gk in range(HKV)]
    wo_s = [np.ascontiguousarray(Wo[:, gk * JL:(gk + 1) * JL].T).astype(bf16)
            for gk in range(HKV)]
    for c in range(8):
        b, gk = c // 4, c % 4
        in_maps.append({
            "xT": xT[b], "wq": wq_s[gk], "wk": wk_s[gk],
            "wv": wv_s[gk], "wo": wo_s[gk],
        })
    return in_maps


def kernel(x, Wq, Wk, Wv, Wo):
    nc = get_nc()
    in_maps = make_in_maps(x, Wq, Wk, Wv, Wo)
    res = run_bass_kernel_spmd(nc, in_maps, core_ids=list(range(8)))
    out = np.empty((B, N, E), np.float32)
    for b in range(B):
        acc = res.results[b * 4]["out"].astype(np.float32)
        for gk in range(1, HKV):
            acc = acc + res.results[b * 4 + gk]["out"].astype(np.float32)
        out[b] = acc
    return out
